# revision 8
# baseline (speedup 1.0000x reference)
# Multi-head attention (B=4, T=2048, C=1024, H=16, D=64) on 8 trn2 NeuronCores.
#
# Sharding: 64 (batch, head) pairs -> 8 per core. Core c handles batch c//2,
# heads 8*(c%2) .. 8*(c%2)+8, i.e. a contiguous [2048, 512] column slice of x
# (and of the output). Q/K/V weights are tiny and replicated (pre-processed on
# host into block-diagonal lhsT form so two heads share one 128-contraction).
#
# Per-core pipeline (heads processed in pairs A,B = one 128-channel block):
#   1. xT = transpose(x-slice) via PE transpose   [128 c, 16 to, 128 t]
#   2. QT2 = wq2.T @ xT2 (+bq), KT2 likewise      [128 e2, 2048 t]  (e2 = eA|eB)
#   3. V2  = xT2.T @ wv2                          [2048 s, eA|eB], ones col 64
#   4. flash loop over 16 key tiles (si) x 4 query chunks (ch):
#        S.T tile = KT2_h.T @ QT2_h   (row-packed pair, fp32r, PSUM [128,1024])
#        P.T = exp(S.T * 0.125)       (ScalarE, PSUM->SBUF; no max-subtraction:
#                                      scores ~ N(0,1), exp can't overflow)
#        O.T[65,512] += [V|1].T @ P.T (fp32r; row 64 accumulates the softmax
#                                      denominator d[t] for free)
#   5. transpose [65,128] chunks -> [128,65]; r = 1/d; out = O*r + bv; DMA out.
import numpy as np

B, T, C = 4, 2048, 1024
H, D = 16, 64
NCORES = 8
PCOLS = C // 2          # 512 columns per core
TO = T // 128           # 16 t tiles
NPAIR = PCOLS // 128    # 4 head pairs per core

_cached_nc = None


def _build_nc():
    import concourse.bass as bass
    import concourse.mybir as mybir
    import concourse.tile as tile
    from concourse import bacc
    from concourse.masks import make_identity

    f32 = mybir.dt.float32
    f32r = mybir.dt.float32r
    AF = mybir.ActivationFunctionType
    ALU = mybir.AluOpType

    nc = bacc.Bacc("TRN2", target_bir_lowering=False, debug=False)

    xs = nc.dram_tensor("xs", [T, PCOLS], f32, kind="ExternalInput")
    wq2 = nc.dram_tensor("wq2", [128, 128], f32, kind="ExternalInput")
    wk2 = nc.dram_tensor("wk2", [128, 128], f32, kind="ExternalInput")
    wv2 = nc.dram_tensor("wv2", [128, 128], f32, kind="ExternalInput")
    bq2 = nc.dram_tensor("bq2", [128, 1], f32, kind="ExternalInput")
    bk2 = nc.dram_tensor("bk2", [128, 1], f32, kind="ExternalInput")
    bvb = nc.dram_tensor("bvb", [128, 64], f32, kind="ExternalInput")
    ys = nc.dram_tensor("ys", [T, PCOLS], f32, kind="ExternalOutput")

    x_r = xs[:].rearrange("(to p) c -> p to c", p=128)   # [128, 16, 512]
    y_r = ys[:].rearrange("(to p) c -> p to c", p=128)   # [128, 16, 512]

    with tile.TileContext(nc) as tc:
        from contextlib import ExitStack

        with ExitStack() as ctx:
            const = ctx.enter_context(tc.tile_pool(name="const", bufs=1))
            xpool = ctx.enter_context(tc.tile_pool(name="xpool", bufs=2))
            xtp = ctx.enter_context(tc.tile_pool(name="xtp", bufs=2))
            qkp = ctx.enter_context(tc.tile_pool(name="qkp", bufs=2))
            vp = ctx.enter_context(tc.tile_pool(name="vp", bufs=2))
            ptp = ctx.enter_context(tc.tile_pool(name="ptp", bufs=2))
            stp = ctx.enter_context(tc.tile_pool(name="stp", bufs=2))
            osp = ctx.enter_context(tc.tile_pool(name="osp", bufs=2))
            smallp = ctx.enter_context(tc.tile_pool(name="smallp", bufs=4))
            # PSUM: tags s(2x2 banks) + oA(1) + oB(1) + pT(2x1) = 8 banks
            ps_s = ctx.enter_context(tc.tile_pool(name="ps_s", bufs=2, space="PSUM"))
            ps_o = ctx.enter_context(tc.tile_pool(name="ps_o", bufs=1, space="PSUM"))
            ps_t = ctx.enter_context(tc.tile_pool(name="ps_t", bufs=2, space="PSUM"))

            ident = const.tile([128, 128], f32)
            make_identity(nc, ident)
            # Dummy PE transpose so the PE observes gpsimd's identity write
            # here; otherwise the first real transpose needs two semaphore
            # waits (gpsimd + DMA) and walrus allows only one on the
            # transpose-mode LDWEIGHTS struct.
            pst0 = ps_t.tile([128, 128], f32, tag="pT")
            nc.tensor.transpose(pst0, ident, ident)
            wq2_sb = const.tile([128, 128], f32)
            wk2_sb = const.tile([128, 128], f32)
            wv2_sb = const.tile([128, 128], f32)
            wq2_r = const.tile([128, 128], f32r)
            wk2_r = const.tile([128, 128], f32r)
            wv2_r = const.tile([128, 128], f32r)
            bq2_sb = const.tile([128, 1], f32)
            bk2_sb = const.tile([128, 1], f32)
            bvb_sb = const.tile([128, 64], f32)
            ones16 = const.tile([128, TO, 1], f32)
            nc.vector.memset(ones16[:], 1.0)
            nc.sync.dma_start(wq2_sb[:], wq2[:])
            nc.sync.dma_start(wk2_sb[:], wk2[:])
            nc.sync.dma_start(wv2_sb[:], wv2[:])
            nc.sync.dma_start(bq2_sb[:], bq2[:])
            nc.sync.dma_start(bk2_sb[:], bk2[:])
            nc.sync.dma_start(bvb_sb[:], bvb[:])
            nc.vector.tensor_copy(wq2_r[:], wq2_sb[:])
            nc.vector.tensor_copy(wk2_r[:], wk2_sb[:])
            nc.vector.tensor_copy(wv2_r[:], wv2_sb[:])

            for p in range(NPAIR):
                # ---- load x column block, transpose -> xT2 [128 c2, to, t] --
                xp = xpool.tile([128, TO, 128], f32, tag="xp")
                nc.sync.dma_start(xp[:], x_r[:, :, p * 128:(p + 1) * 128])
                xT2 = xtp.tile([128, TO, 128], f32r, tag="xT2")
                for to in range(TO):
                    pst = ps_t.tile([128, 128], f32, tag="pT")
                    nc.tensor.transpose(pst, xp[:, to, :], ident)
                    nc.vector.tensor_copy(xT2[:, to, :], pst)

                # ---- projections QT2, KT2 [128 e2, 16 to, 128 t] ----
                QT2 = qkp.tile([128, TO, 128], f32r, tag="qt")
                KT2 = qkp.tile([128, TO, 128], f32r, tag="kt")
                for ch in range(4):
                    rhs = xT2[:, 4 * ch:4 * ch + 4, :]
                    psq = ps_s.tile([128, 512], f32, tag="s")
                    nc.tensor.matmul(psq, wq2_r[:], rhs,
                                     start=True, stop=True)
                    nc.scalar.activation(QT2[:, 4 * ch:4 * ch + 4, :], psq,
                                         AF.Identity, bias=bq2_sb[:])
                    psk = ps_s.tile([128, 512], f32, tag="s")
                    nc.tensor.matmul(psk, wk2_r[:], rhs,
                                     start=True, stop=True)
                    nc.scalar.activation(KT2[:, 4 * ch:4 * ch + 4, :], psk,
                                         AF.Identity, bias=bk2_sb[:])

                # ---- V2A/V2B [128 s, 16 to, 65] with ones in col 64 ----
                V2A = vp.tile([128, TO, 65], f32r, tag="vA")
                V2B = vp.tile([128, TO, 65], f32r, tag="vB")
                nc.vector.tensor_copy(V2A[:, :, 64:65], ones16[:])
                nc.vector.tensor_copy(V2B[:, :, 64:65], ones16[:])
                for to in range(TO):
                    psv = ps_t.tile([128, 128], f32, tag="pT")
                    nc.tensor.matmul(psv, xT2[:, to, :], wv2_r[:],
                                     start=True, stop=True)
                    nc.vector.tensor_copy(V2A[:, to, 0:64], psv[:, 0:64])
                    nc.vector.tensor_copy(V2B[:, to, 0:64], psv[:, 64:128])

                # ---- attention ----
                for ch in range(4):
                    oA = ps_o.tile([65, 512], f32, tag="oA")
                    oB = ps_o.tile([65, 512], f32, tag="oB")
                    for si in range(TO):
                        sAB = ps_s.tile([128, 1024], f32, tag="s")
                        nc.tensor.matmul(
                            sAB[:, 0:512],
                            KT2[0:64, si, :],
                            QT2[0:64, 4 * ch:4 * ch + 4, :],
                            start=True, stop=True)
                        nc.tensor.matmul(
                            sAB[:, 512:1024],
                            KT2[64:128, si, :],
                            QT2[64:128, 4 * ch:4 * ch + 4, :],
                            start=True, stop=True)
                        ptAB = ptp.tile([128, 1024], f32r, tag="pt")
                        nc.scalar.activation(ptAB, sAB, AF.Exp, scale=0.125)
                        nc.tensor.matmul(
                            oA, V2A[:, si, :],
                            ptAB[:, 0:512],
                            start=(si == 0), stop=(si == TO - 1))
                        nc.tensor.matmul(
                            oB, V2B[:, si, :],
                            ptAB[:, 512:1024],
                            start=(si == 0), stop=(si == TO - 1))

                    stA = stp.tile([65, 512], f32, tag="stA")
                    stB = stp.tile([65, 512], f32, tag="stB")
                    nc.vector.tensor_copy(stA[:], oA)
                    nc.vector.tensor_copy(stB[:], oB)

                    ost = osp.tile([128, 4, 128], f32, tag="ost")
                    for k in range(4):
                        pTA = ps_t.tile([128, 65], f32, tag="pT")
                        nc.tensor.transpose(
                            pTA, stA[:, k * 128:(k + 1) * 128],
                            ident[0:65, 0:65])
                        rA = smallp.tile([128, 1], f32, tag="r")
                        nc.vector.reciprocal(rA, pTA[:, 64:65])
                        nc.vector.scalar_tensor_tensor(
                            out=ost[:, k, 0:64], in0=pTA[:, 0:64],
                            scalar=rA[:], in1=bvb_sb[:],
                            op0=ALU.mult, op1=ALU.add)
                        pTB = ps_t.tile([128, 65], f32, tag="pT")
                        nc.tensor.transpose(
                            pTB, stB[:, k * 128:(k + 1) * 128],
                            ident[0:65, 0:65])
                        rB = smallp.tile([128, 1], f32, tag="r")
                        nc.vector.reciprocal(rB, pTB[:, 64:65])
                        nc.vector.scalar_tensor_tensor(
                            out=ost[:, k, 64:128], in0=pTB[:, 0:64],
                            scalar=rB[:], in1=bvb_sb[:],
                            op0=ALU.mult, op1=ALU.add)
                    nc.sync.dma_start(
                        y_r[:, 4 * ch:4 * ch + 4, p * 128:(p + 1) * 128],
                        ost[:])
    nc.compile()
    return nc


def _host_inputs(x, Wq, bq, Wk, bk, Wv, bv):
    def blockdiag(w):
        out = np.zeros((128, 128), dtype=np.float32)
        out[0:64, 0:64] = w
        out[64:128, 64:128] = w
        return out

    wq2 = blockdiag(np.ascontiguousarray(Wq.T))
    wk2 = blockdiag(np.ascontiguousarray(Wk.T))
    wv2 = blockdiag(np.ascontiguousarray(Wv.T))
    bq2 = np.concatenate([bq, bq]).reshape(128, 1).astype(np.float32)
    bk2 = np.concatenate([bk, bk]).reshape(128, 1).astype(np.float32)
    bvb = np.tile(bv.reshape(1, 64), (128, 1)).astype(np.float32)

    in_maps = []
    for c in range(NCORES):
        b, half = c // 2, c % 2
        xsl = np.ascontiguousarray(x[b, :, half * PCOLS:(half + 1) * PCOLS],
                                   dtype=np.float32)
        in_maps.append({
            "xs": xsl, "wq2": wq2, "wk2": wk2, "wv2": wv2,
            "bq2": bq2, "bk2": bk2, "bvb": bvb,
        })
    return in_maps


def _run(x, Wq, bq, Wk, bk, Wv, bv, trace=False):
    from concourse.bass_utils import run_bass_kernel_spmd

    global _cached_nc
    if _cached_nc is None:
        _cached_nc = _build_nc()
    in_maps = _host_inputs(x, Wq, bq, Wk, bk, Wv, bv)
    res = run_bass_kernel_spmd(_cached_nc, in_maps,
                               core_ids=list(range(NCORES)), trace=trace)
    y = np.empty((B, T, C), dtype=np.float32)
    for c in range(NCORES):
        b, half = c // 2, c % 2
        y[b, :, half * PCOLS:(half + 1) * PCOLS] = res.results[c]["ys"]
    return y, res


def kernel(x, Wq, bq, Wk, bk, Wv, bv):
    y, _ = _run(np.asarray(x), np.asarray(Wq), np.asarray(bq), np.asarray(Wk),
                np.asarray(bk), np.asarray(Wv), np.asarray(bv))
    return y


# revision 10
# speedup vs baseline: 3.5543x; 3.5543x over previous
# Multi-head attention (B=4, T=2048, C=1024, H=16, D=64) on 8 trn2 NeuronCores.
#
# Sharding: 64 (batch, head) pairs -> 8 per core. Core c handles batch c//2,
# heads 8*(c%2) .. 8*(c%2)+8, i.e. a contiguous [2048, 512] column slice of x
# (and of the output). Q/K/V weights are tiny and replicated (pre-processed on
# host into block-diagonal lhsT form so two heads share one 128-contraction).
#
# Per-core pipeline (heads processed in pairs A,B = one 128-channel block):
#   1. xT = transpose(x-slice) via PE transpose   [128 c, 16 to, 128 t]
#   2. QT2 = wq2.T @ xT2 (+bq), KT2 likewise      [128 e2, 2048 t]  (e2 = eA|eB)
#   3. V2  = xT2.T @ wv2                          [2048 s, eA|eB], ones col 64
#   4. flash loop over 16 key tiles (si) x 4 query chunks (ch):
#        S.T tile = KT2_h.T @ QT2_h   (row-packed pair, fp32r, PSUM [128,1024])
#        P.T = exp(S.T * 0.125)       (ScalarE, PSUM->SBUF; no max-subtraction:
#                                      scores ~ N(0,1), exp can't overflow)
#        O.T[65,512] += [V|1].T @ P.T (fp32r; row 64 accumulates the softmax
#                                      denominator d[t] for free)
#   5. transpose [65,128] chunks -> [128,65]; r = 1/d; out = O*r + bv; DMA out.
import numpy as np

B, T, C = 4, 2048, 1024
H, D = 16, 64
NCORES = 8
PCOLS = C // 2          # 512 columns per core
TO = T // 128           # 16 t tiles
NPAIR = PCOLS // 128    # 4 head pairs per core

_cached_nc = None


def _build_nc(reps=1):
    import concourse.bass as bass
    import concourse.mybir as mybir
    import concourse.tile as tile
    from concourse import bacc
    from concourse.masks import make_identity

    f32 = mybir.dt.float32
    f32r = mybir.dt.float32r
    AF = mybir.ActivationFunctionType
    ALU = mybir.AluOpType

    nc = bacc.Bacc("TRN2", target_bir_lowering=False, debug=False)

    xs = nc.dram_tensor("xs", [T, PCOLS], f32, kind="ExternalInput")
    wq2 = nc.dram_tensor("wq2", [128, 128], f32, kind="ExternalInput")
    wk2 = nc.dram_tensor("wk2", [128, 128], f32, kind="ExternalInput")
    wv2 = nc.dram_tensor("wv2", [128, 256], f32, kind="ExternalInput")
    bq2 = nc.dram_tensor("bq2", [128, 1], f32, kind="ExternalInput")
    bk2 = nc.dram_tensor("bk2", [128, 1], f32, kind="ExternalInput")
    bvb = nc.dram_tensor("bvb", [128, 64], f32, kind="ExternalInput")
    ys = nc.dram_tensor("ys", [T, PCOLS], f32, kind="ExternalOutput")

    x_r = xs[:].rearrange("(to p) c -> p to c", p=128)   # [128, 16, 512]
    y_r = ys[:].rearrange("(to p) c -> p to c", p=128)   # [128, 16, 512]

    with tile.TileContext(nc) as tc:
        from contextlib import ExitStack

        with ExitStack() as ctx:
            const = ctx.enter_context(tc.tile_pool(name="const", bufs=1))
            xpool = ctx.enter_context(tc.tile_pool(name="xpool", bufs=2))
            xtp = ctx.enter_context(tc.tile_pool(name="xtp", bufs=2))
            qkp = ctx.enter_context(tc.tile_pool(name="qkp", bufs=2))
            vp = ctx.enter_context(tc.tile_pool(name="vp", bufs=2))
            ptp = ctx.enter_context(tc.tile_pool(name="ptp", bufs=2))
            stp = ctx.enter_context(tc.tile_pool(name="stp", bufs=2))
            osp = ctx.enter_context(tc.tile_pool(name="osp", bufs=2))
            smallp = ctx.enter_context(tc.tile_pool(name="smallp", bufs=4))
            # PSUM: tags s(2x2 banks) + oA(1) + oB(1) + pT(2x1) = 8 banks
            ps_s = ctx.enter_context(tc.tile_pool(name="ps_s", bufs=2, space="PSUM"))
            ps_o = ctx.enter_context(tc.tile_pool(name="ps_o", bufs=1, space="PSUM"))
            ps_t = ctx.enter_context(tc.tile_pool(name="ps_t", bufs=2, space="PSUM"))

            ident = const.tile([128, 128], f32)
            make_identity(nc, ident)
            # Dummy PE transpose so the PE observes gpsimd's identity write
            # here; otherwise the first real transpose needs two semaphore
            # waits (gpsimd + DMA) and walrus allows only one on the
            # transpose-mode LDWEIGHTS struct.
            pst0 = ps_t.tile([128, 128], f32, tag="pT")
            nc.tensor.transpose(pst0, ident, ident)
            wq2_sb = const.tile([128, 128], f32)
            wk2_sb = const.tile([128, 128], f32)
            wv2_sb = const.tile([128, 256], f32)
            wq2_r = const.tile([128, 128], f32r)
            wk2_r = const.tile([128, 128], f32r)
            wv2_r = const.tile([128, 256], f32r)
            bq2_sb = const.tile([128, 1], f32)
            bk2_sb = const.tile([128, 1], f32)
            bvb_sb = const.tile([128, 64], f32)
            ones16 = const.tile([128, TO, 1], f32)
            nc.vector.memset(ones16[:], 1.0)
            ones_r = const.tile([128, 1], f32r)
            nc.vector.tensor_copy(ones_r[:], ones16[:, 0, :])
            nc.sync.dma_start(wq2_sb[:], wq2[:])
            nc.sync.dma_start(wk2_sb[:], wk2[:])
            nc.sync.dma_start(wv2_sb[:], wv2[:])
            nc.sync.dma_start(bq2_sb[:], bq2[:])
            nc.sync.dma_start(bk2_sb[:], bk2[:])
            nc.sync.dma_start(bvb_sb[:], bvb[:])
            nc.vector.tensor_copy(wq2_r[:], wq2_sb[:])
            nc.vector.tensor_copy(wk2_r[:], wk2_sb[:])
            nc.vector.tensor_copy(wv2_r[:], wv2_sb[:])

            import contextlib
            loop_cm = tc.For_i(0, reps, 1) if reps > 1 else \
                contextlib.nullcontext()
            with loop_cm:
              for p in range(NPAIR):
                # ---- load x column block, transpose -> xT2 [128 c2, to, t] --
                xp = xpool.tile([128, TO, 128], f32, tag="xp")
                nc.sync.dma_start(xp[:], x_r[:, :, p * 128:(p + 1) * 128])
                xT2 = xtp.tile([128, TO, 128], f32r, tag="xT2")
                for to in range(TO):
                    pst = ps_t.tile([128, 128], f32, tag="pT")
                    nc.tensor.transpose(pst, xp[:, to, :], ident)
                    nc.vector.tensor_copy(xT2[:, to, :], pst)

                # ---- projections QT2, KT2 [128 e2, 16 to, 128 t] ----
                QT2 = qkp.tile([128, TO, 128], f32r, tag="qt")
                KT2 = qkp.tile([128, TO, 128], f32r, tag="kt")
                for ch in range(4):
                    rhs = xT2[:, 4 * ch:4 * ch + 4, :]
                    psq = ps_s.tile([128, 512], f32, tag="s")
                    nc.tensor.matmul(psq, wq2_r[:], rhs,
                                     start=True, stop=True)
                    nc.scalar.activation(QT2[:, 4 * ch:4 * ch + 4, :], psq,
                                         AF.Identity, bias=bq2_sb[:])
                    psk = ps_s.tile([128, 512], f32, tag="s")
                    nc.tensor.matmul(psk, wk2_r[:], rhs,
                                     start=True, stop=True)
                    nc.scalar.activation(KT2[:, 4 * ch:4 * ch + 4, :], psk,
                                         AF.Identity, bias=bk2_sb[:])

                # ---- V2A/V2B [128 s, 16 to, 65] with ones in col 64 ----
                V2A = vp.tile([128, TO, 64], f32r, tag="vA")
                V2B = vp.tile([128, TO, 64], f32r, tag="vB")
                for to in range(TO):
                    psv = ps_t.tile([128, 256], f32, tag="pT")
                    nc.tensor.matmul(psv, xT2[:, to, :], wv2_r[:],
                                     start=True, stop=True)
                    nc.vector.tensor_copy(V2A[:, to, :], psv[:, 0:64])
                    nc.vector.tensor_copy(V2B[:, to, :], psv[:, 64:128])

                # ---- attention ----
                for ch in range(4):
                    oAB = ps_o.tile([128, 512], f32, tag="oAB")
                    dAB = ps_o.tile([128, 512], f32, tag="dAB")
                    for si in range(TO):
                        sAB = ps_s.tile([128, 1024], f32, tag="s")
                        nc.tensor.matmul(
                            sAB[:, 0:512],
                            KT2[0:64, si, :],
                            QT2[0:64, 4 * ch:4 * ch + 4, :],
                            start=True, stop=True)
                        nc.tensor.matmul(
                            sAB[:, 512:1024],
                            KT2[64:128, si, :],
                            QT2[64:128, 4 * ch:4 * ch + 4, :],
                            start=True, stop=True)
                        ptAB = ptp.tile([128, 1024], f32r, tag="pt")
                        nc.scalar.activation(ptAB, sAB, AF.Exp, scale=0.125)
                        nc.tensor.matmul(
                            oAB[0:64, :], V2A[:, si, :],
                            ptAB[:, 0:512],
                            start=(si == 0), stop=(si == TO - 1),
                            tile_position=(0, 0))
                        nc.tensor.matmul(
                            oAB[64:128, :], V2B[:, si, :],
                            ptAB[:, 512:1024],
                            start=(si == 0), stop=(si == TO - 1),
                            tile_position=(0, 64))
                        nc.tensor.matmul(
                            dAB[0:1, :], ones_r[:],
                            ptAB[:, 0:512],
                            start=(si == 0), stop=(si == TO - 1),
                            tile_position=(0, 0))
                        nc.tensor.matmul(
                            dAB[32:33, :], ones_r[:],
                            ptAB[:, 512:1024],
                            start=(si == 0), stop=(si == TO - 1),
                            tile_position=(0, 32))

                    stA = stp.tile([65, 512], f32, tag="stA")
                    stB = stp.tile([65, 512], f32, tag="stB")
                    nc.vector.tensor_copy(stA[0:64, :], oAB[0:64, :])
                    nc.vector.tensor_copy(stA[64:65, :], dAB[0:1, :])
                    nc.vector.tensor_copy(stB[0:64, :], oAB[64:128, :])
                    nc.vector.tensor_copy(stB[64:65, :], dAB[32:33, :])

                    ost = osp.tile([128, 4, 128], f32, tag="ost")
                    for k in range(4):
                        pTA = ps_t.tile([128, 65], f32, tag="pT")
                        nc.tensor.transpose(
                            pTA, stA[:, k * 128:(k + 1) * 128],
                            ident[0:65, 0:65])
                        rA = smallp.tile([128, 1], f32, tag="r")
                        nc.vector.reciprocal(rA, pTA[:, 64:65])
                        nc.vector.scalar_tensor_tensor(
                            out=ost[:, k, 0:64], in0=pTA[:, 0:64],
                            scalar=rA[:], in1=bvb_sb[:],
                            op0=ALU.mult, op1=ALU.add)
                        pTB = ps_t.tile([128, 65], f32, tag="pT")
                        nc.tensor.transpose(
                            pTB, stB[:, k * 128:(k + 1) * 128],
                            ident[0:65, 0:65])
                        rB = smallp.tile([128, 1], f32, tag="r")
                        nc.vector.reciprocal(rB, pTB[:, 64:65])
                        nc.vector.scalar_tensor_tensor(
                            out=ost[:, k, 64:128], in0=pTB[:, 0:64],
                            scalar=rB[:], in1=bvb_sb[:],
                            op0=ALU.mult, op1=ALU.add)
                    nc.sync.dma_start(
                        y_r[:, 4 * ch:4 * ch + 4, p * 128:(p + 1) * 128],
                        ost[:])
    nc.compile()
    return nc


def _host_inputs(x, Wq, bq, Wk, bk, Wv, bv):
    def blockdiag(w):
        out = np.zeros((128, 128), dtype=np.float32)
        out[0:64, 0:64] = w
        out[64:128, 64:128] = w
        return out

    wq2 = blockdiag(np.ascontiguousarray(Wq.T))
    wk2 = blockdiag(np.ascontiguousarray(Wk.T))
    wv2_1 = blockdiag(np.ascontiguousarray(Wv.T))
    wv2 = np.concatenate([wv2_1, wv2_1], axis=1)
    bq2 = np.concatenate([bq, bq]).reshape(128, 1).astype(np.float32)
    bk2 = np.concatenate([bk, bk]).reshape(128, 1).astype(np.float32)
    bvb = np.tile(bv.reshape(1, 64), (128, 1)).astype(np.float32)

    in_maps = []
    for c in range(NCORES):
        b, half = c // 2, c % 2
        xsl = np.ascontiguousarray(x[b, :, half * PCOLS:(half + 1) * PCOLS],
                                   dtype=np.float32)
        in_maps.append({
            "xs": xsl, "wq2": wq2, "wk2": wk2, "wv2": wv2,
            "bq2": bq2, "bk2": bk2, "bvb": bvb,
        })
    return in_maps


def _run(x, Wq, bq, Wk, bk, Wv, bv, trace=False):
    from concourse.bass_utils import run_bass_kernel_spmd

    global _cached_nc
    if _cached_nc is None:
        _cached_nc = _build_nc()
    in_maps = _host_inputs(x, Wq, bq, Wk, bk, Wv, bv)
    res = run_bass_kernel_spmd(_cached_nc, in_maps,
                               core_ids=list(range(NCORES)), trace=trace)
    y = np.empty((B, T, C), dtype=np.float32)
    for c in range(NCORES):
        b, half = c // 2, c % 2
        y[b, :, half * PCOLS:(half + 1) * PCOLS] = res.results[c]["ys"]
    return y, res


def kernel(x, Wq, bq, Wk, bk, Wv, bv):
    y, _ = _run(np.asarray(x), np.asarray(Wq), np.asarray(bq), np.asarray(Wk),
                np.asarray(bk), np.asarray(Wv), np.asarray(bv))
    return y
